# revision 10
# baseline (speedup 1.0000x reference)
"""NeighborhoodShift2d: stack 49 spatially shifted (zero-padded) copies.

Input  x:  [1, 8, 32, 128, 128]  (B, heads, dim, H, W) fp32
Output y:  [1, 8, 49, 32, 128, 128]  y[:, :, k] = shift(x, OFFSETS[k]) with
zero padding, k enumerating the 7x7 NATTEN stencil (dy major, dx minor).

Sharding: pure data-parallel, one head per NeuronCore (8 heads, 8 cores).

Per-core program (pure DMA, memory-bound). Design notes:
- SDMA throughput is per-descriptor-bound, so every transfer uses big
  contiguous descriptors (up to 64 KiB). The dx shift is baked into SBUF
  as 7 flat per-channel image copies, each loaded DIRECTLY from DRAM as a
  flat shifted window (x[c].flat[dx:FP] is contiguous!). The |dx| wrap
  columns (row-boundary wrap garbage / dx zero padding) are then zeroed
  by DVE memsets. A store descriptor is a fully contiguous
  (H-|dy|)*W-float run per channel.
- SBUF AXI port parity: partitions [0,64) sit on the 8 even ports,
  [64,128) on the 8 odd ports (~218 GB/s per parity). Bands are placed
  so each parity carries half the store traffic; the sync (SP) queue
  issues only even-parity-band stores, scalar (ACT) only odd. The dx=0
  image is kept twice (T1[96:128] odd, T2[0:32] even) and its stores
  alternate parity by dy.
- T1 bands (by partition/32): [-3, -2, -1, 0dup]; T2: [0dup, +1, +2, +3]
- Stores are gated per band (semaphore thresholds) so they start as soon
  as that band's load+memset landed, not after the whole prologue.
- Edge rows (|dy| rows outside the image) are zero-filled from a zero
  tile with one batched DMA per dy (all 7 k-blocks at once).
"""

import numpy as np

import concourse.bass as bass
import concourse.mybir as mybir
from concourse.bass_utils import run_bass_kernel_spmd

B, HEADS, C, H, W = 1, 8, 32, 128, 128
WIN = 7
PAD = 3
K = WIN * WIN
FP = H * W  # flat image floats per partition (16384)

_nc_cache = None


def _build_nc():
    f32 = mybir.dt.float32
    nc = bass.Bass()
    x = nc.dram_tensor("x", [C, H, W], f32, kind="ExternalInput")
    y = nc.dram_tensor("y", [K, C, H, W], f32, kind="ExternalOutput")

    with (
        nc.sbuf_tensor("T1", [4 * C, H, W], f32) as T1,
        nc.sbuf_tensor("T2", [4 * C, H, W], f32) as T2,
        nc.sbuf_tensor("Z", [112, 768], f32) as Z,
        nc.semaphore("s_ldS") as s_ldS,
        nc.semaphore("s_ldA") as s_ldA,
        nc.semaphore("s_dve") as s_dve,
        nc.semaphore("s_sp") as s_sp,
        nc.semaphore("s_act") as s_act,
        nc.Block() as block,
    ):
        def band(dx, dy=0):
            """(tensor, first partition) of the dx-shifted flat copy."""
            if dx < 0:
                return T1, 32 * (dx + 3)
            if dx > 0:
                return T2, 32 * dx
            return (T2, 0) if dy % 2 == 0 else (T1, 96)

        # s_dve thresholds at which each band's wrap memset has landed
        BAND_READY = {1: 2, -1: 3, -2: 4, 2: 5, -3: 6, 3: 7}

        def load_band(eng, dx, sem, dup_parity=None):
            """Flat (shifted) load of the whole head into band(dx)."""
            if dx == 0:
                buf, p0 = (T2, 0) if dup_parity == "even" else (T1, 96)
            else:
                buf, p0 = band(dx)
            xf = x.rearrange("c h w -> c (h w)")
            if dx >= 0:
                dst = bass.AP(buf, p0 * FP, [[FP, C], [1, FP - dx]])
                src = xf[:, dx:FP]
            else:
                dst = bass.AP(buf, p0 * FP - dx, [[FP, C], [1, FP + dx]])
                src = xf[:, 0 : FP + dx]
            eng.dma_start(out=dst, in_=src).then_inc(sem, 16)

        def edge(eng, dy, sem):
            g = abs(dy)
            k0 = (dy + PAD) * WIN
            r0 = 0 if dy < 0 else H - g
            eng.dma_start(
                out=y[k0 : k0 + WIN, :, r0 : r0 + g, :],
                in_=bass.AP(Z, 0, [[768, 112], [384, 2], [1, 128 * g]]),
            ).then_inc(sem, 16)

        def store(eng, dy, dx, sem):
            n = H - abs(dy)
            ys, yd = max(0, dy), max(0, -dy)
            k = (dy + PAD) * WIN + (dx + PAD)
            buf, p0 = band(dx, dy)
            src = bass.AP(buf, p0 * FP + ys * W, [[FP, C], [1, n * W]])
            dst = y[k, :, yd : yd + n, :]
            eng.dma_start(out=dst, in_=src).then_inc(sem, 16)

        @block.vector
        def _(vector):
            vector.memset(Z[:, :], 0.0).then_inc(s_dve, 1)
            # Gate each band's wrap-column memset on its own load.
            # sync loads T2 bands [dup-e, +1, +2, +3] (incs s_ldA 16..64);
            # scalar loads T1 bands [dup-o, -1, -2, -3] (incs s_ldS 16..64).
            for dx, sem, thr in (
                (1, s_ldA, 32),
                (-1, s_ldS, 32),
                (-2, s_ldS, 48),
                (2, s_ldA, 48),
                (-3, s_ldS, 64),
                (3, s_ldA, 64),
            ):
                vector.wait_ge(sem, thr)
                buf, p0 = band(dx)
                if dx < 0:
                    ap = buf[p0 : p0 + C, :, 0:-dx]
                else:
                    ap = buf[p0 : p0 + C, :, W - dx : W]
                vector.memset(ap, 0.0).then_inc(s_dve, 1)

        dys = list(range(-PAD, PAD + 1))

        # Each queue loads the bands it stores, just-in-time between store
        # bursts, so stores saturate the rings from the start.
        @block.sync
        def _(sync):
            n_st = 0

            def burst(dx):
                nonlocal n_st
                for dy in dys:
                    if dx != 0 or dy % 2 == 0:
                        store(nc.sync, dy, dx if dx != 0 else 0, s_sp)
                        n_st += 1

            load_band(nc.sync, 0, s_ldA, dup_parity="even")   # s_ldA 16
            load_band(nc.sync, 1, s_ldA)                      # s_ldA 32
            sync.wait_ge(s_dve, 1)
            for dy in (-3, -2, -1):
                edge(nc.sync, dy, s_sp)
            sync.wait_ge(s_ldA, 16)
            burst(0)                      # dup-even: dy in {-2, 0, 2}
            load_band(nc.sync, 2, s_ldA)                      # s_ldA 48
            sync.wait_ge(s_dve, BAND_READY[1])
            burst(1)
            load_band(nc.sync, 3, s_ldA)                      # s_ldA 64
            sync.wait_ge(s_dve, BAND_READY[-2])
            burst(-2)
            sync.wait_ge(s_dve, BAND_READY[-3])
            burst(-3)
            sync.wait_ge(s_sp, 16 * (3 + n_st))

        @block.scalar
        def _(scalar):
            n_st = 0

            def burst(dx):
                nonlocal n_st
                for dy in dys:
                    if dx != 0 or dy % 2 != 0:
                        store(nc.scalar, dy, dx if dx != 0 else 0, s_act)
                        n_st += 1

            load_band(nc.scalar, 0, s_ldS, dup_parity="odd")  # s_ldS 16
            load_band(nc.scalar, -1, s_ldS)                   # s_ldS 32
            scalar.wait_ge(s_dve, 1)
            for dy in (1, 2, 3):
                edge(nc.scalar, dy, s_act)
            scalar.wait_ge(s_ldS, 16)
            burst(0)                      # dup-odd: dy in {-3, -1, 1, 3}
            load_band(nc.scalar, -2, s_ldS)                   # s_ldS 48
            scalar.wait_ge(s_dve, BAND_READY[-1])
            burst(-1)
            load_band(nc.scalar, -3, s_ldS)                   # s_ldS 64
            scalar.wait_ge(s_dve, BAND_READY[2])
            burst(2)
            scalar.wait_ge(s_dve, BAND_READY[3])
            burst(3)
            scalar.wait_ge(s_act, 16 * (3 + n_st))

    return nc


def _get_nc():
    global _nc_cache
    if _nc_cache is None:
        _nc_cache = _build_nc()
    return _nc_cache


def kernel(x: np.ndarray) -> np.ndarray:
    assert x.shape == (B, HEADS, C, H, W), x.shape
    nc = _get_nc()
    in_maps = [
        {"x": np.ascontiguousarray(x[0, h], dtype=np.float32)} for h in range(HEADS)
    ]
    res = run_bass_kernel_spmd(nc, in_maps, core_ids=list(range(HEADS)))
    out = np.stack([res.results[h]["y"] for h in range(HEADS)], axis=0)
    return out[None]  # [1, 8, 49, 32, 128, 128]
